# revision 41
# baseline (speedup 1.0000x reference)
"""Multi-head attention (16 heads, D=1024, B=2, S=2048) on 8 TRN2 NeuronCores.

Sharding: tensor-parallel over heads. Each core owns 2 heads (128 features):
W_q/k/v column-sliced, W_o row-sliced; partial outputs summed on host.

Device dataflow (per core):
  QT[f,s], KT[f,s] = W^T x^T   (feat-major projections, contraction on parts)
  V[k,f]           = x W       (token-major projection, k on partitions)
  scores^T[k,q] = KT_h^T . QT_h  per 128-k chunk, both heads -> one psum pair
  attn^T = exp(scores*0.125 + mask_bias[k])  (ACT, psum->SBUF fp16)
  av[q, 65] += attn_chunk_h^T . [V_h | 1]    (N=65 matmuls, psum accum over k;
        col 64 = softmax denominator, per-partition in q!)
  recip = 1/rowsum (DVE), attn_out[q,f] = av * recip  (tensor_scalar per-part)
  outT[f,q] via PE transpose;  out_part[s,:] = outT^T . Wo  -> fp16, host sums

Key-padding mask: k-chunks beyond valid_len are skipped (program specialized
on valid_lens); boundary chunk masked via -1e6 exp bias (underflows to 0).

Build order = schedule: engines run in program order, so proj(b+1) groups and
the qt Wo-tails are interleaved into the attn chunk loop as PE filler to keep
ACT (the exp bottleneck) saturated.

cfg proj8 (off by default): fp8e4 + DoubleRow projections. Implemented and
numerically exact vs a numpy fp8 simulation, but e4m3 quantization of the
projection inputs costs 5.6e-2 (qk) / 7.4e-2 (qkv) relative error against
the 2e-2 gate, so the default stays fp16 end-to-end (rel err 1.0e-3).
"""

import math
import os

import ml_dtypes
import numpy as np

B = 2
S = 2048
D = 1024
NT = B * S          # 4096 rows, b-major
F = 128             # features per core (2 heads x 64)
DH = 64
P = 128
DK = D // P         # 8 contraction chunks for projections
N_CORES = 8
NEG = -1e6

_CACHE: dict = {}


def _build_program(KC: tuple[int, int], cfg: dict):
    import concourse.bass as bass
    import concourse.tile as tile
    from concourse import mybir
    from concourse.masks import make_identity

    dt = mybir.dt
    DT_IN = getattr(dt, cfg["dt_in"])        # W_o / non-fp8 operands
    DT_ATTN = getattr(dt, cfg["dt_attn"])    # attn / V / QT / KT storage
    DT_OUT = getattr(dt, cfg["dt_out"])      # partial output in HBM
    p8 = str(cfg.get("proj8", "0"))
    F8 = {"q": p8 in ("1", "qk", "q"), "k": p8 in ("1", "qk", "q"),
          "v": p8 == "1"}
    def dtx(t):
        return dt.float8e4 if F8[t] else DT_IN
    PM = mybir.MatmulPerfMode.DoubleRow
    Exp = mybir.ActivationFunctionType.Exp

    nc = bass.Bass("TRN2")

    xtq_d = nc.dram_tensor("xtq", [D, NT], dtx("q"), kind="ExternalInput")
    xtk_d = nc.dram_tensor("xtk", [D, NT], dtx("k"), kind="ExternalInput")
    xtv_d = nc.dram_tensor("xtv", [D, NT], dtx("v"), kind="ExternalInput")
    wq_d = nc.dram_tensor("wq", [D, F], dtx("q"), kind="ExternalInput")
    wk_d = nc.dram_tensor("wk", [D, F], dtx("k"), kind="ExternalInput")
    wv_d = nc.dram_tensor("wv", [D, F], dtx("v"), kind="ExternalInput")
    wo_d = nc.dram_tensor("wo", [F, D], DT_IN, kind="ExternalInput")
    mask_d = nc.dram_tensor("maskt", [P, B * 16], dt.float32, kind="ExternalInput")
    out_d = nc.dram_tensor("out_part", [NT, D], DT_OUT, kind="ExternalOutput")

    KW = [KC[0] * 128, KC[1] * 128]   # K/V token count per batch

    from contextlib import ExitStack

    with tile.TileContext(nc) as tc, ExitStack() as ctx:
        const = ctx.enter_context(tc.tile_pool(name="const", bufs=1))
        xpool = ctx.enter_context(tc.tile_pool(name="xpool", bufs=12))
        apool = ctx.enter_context(tc.tile_pool(name="apool", bufs=4))
        aopool = ctx.enter_context(tc.tile_pool(name="aopool", bufs=5))
        otpool = ctx.enter_context(tc.tile_pool(name="otpool", bufs=2))
        ospool = ctx.enter_context(tc.tile_pool(name="ospool", bufs=2))
        rpool = ctx.enter_context(tc.tile_pool(name="rpool", bufs=4))
        ps_sc = ctx.enter_context(
            tc.tile_pool(name="ps_sc", bufs=2, space="PSUM"))
        ps_av = ctx.enter_context(
            tc.tile_pool(name="ps_av", bufs=1, space="PSUM"))
        ps_wo = ctx.enter_context(
            tc.tile_pool(name="ps_wo", bufs=2, space="PSUM"))

        # ---- constants ----
        # Matmult instructions tolerate only ONE sync-wait; weight/identity
        # loads bounce DRAM -> raw tile -> DVE copy so matmul deps merge.
        def dve_load(dst, src_ap, raw_shape, raw_dtype, nm):
            raw = const.tile(list(raw_shape), raw_dtype, tag=f"{nm}_raw",
                             name=f"{nm}_raw")
            nc.sync.dma_start(raw, src_ap)
            nc.vector.tensor_copy(out=dst, in_=raw)

        def wsh(t):
            return [P, DK // 2, 2, F] if F8[t] else [P, DK, F]

        def wre(wd, t):
            if F8[t]:
                return wd.rearrange("(c two p) f -> p c two f", p=P, two=2)
            return wd.rearrange("(kc p) f -> p kc f", p=P)
        wq_sb = const.tile(wsh("q"), dtx("q"), tag="wq")
        wk_sb = const.tile(wsh("k"), dtx("k"), tag="wk")
        wv_sb = const.tile(wsh("v"), dtx("v"), tag="wv")
        mask_sb = const.tile([P, B * 16], dt.float32, tag="mask")
        # wo / identity are needed only by the first qt tail (~30us in);
        # emitted after the startup x DMAs so they don't delay them
        wo_sb = const.tile([F, D], DT_IN, tag="wo")
        ident = const.tile([P, P], DT_ATTN, tag="ident")

        def late_consts():
            dve_load(wo_sb, wo_d[:, :], [F, D], DT_IN, "wo")
            ident_g = const.tile([P, P], DT_ATTN, tag="ident_g")
            make_identity(nc, ident_g)
            nc.vector.tensor_copy(out=ident, in_=ident_g)

        # PE warmup: a few junk matmuls anchor the p-state ramp so the
        # first projections run at full clock (scratch psum, never read)
        warm = const.tile([P, 512], DT_ATTN, tag="warm")
        nc.vector.memset(warm, 1.0)
        for _ in range(2):
            wps = ps_sc.tile([P, 512], dt.float32, tag="sc", name="wps")
            nc.tensor.matmul(wps, lhsT=warm[:, 0:128], rhs=warm)
        # drain holds the PE sequencer until the warmup completes, so real
        # matmuls dispatch with the p-state ramp already past 3us (the cost
        # model prices matmuls at dispatch time)
        nc.tensor.drain()

        QT = const.tile([P, NT], DT_ATTN, tag="QT")
        KT = const.tile([P, NT], DT_ATTN, tag="KT")
        # V natural layout (k on partitions) per 128-k chunk:
        # cols 0:64 = head0, col 64 = ones, cols 65:129 = head1, col 129 = ones
        V = const.tile([P, B * 16, 130], DT_ATTN, tag="V")
        nc.vector.memset(V[:, :, 64:65], 1.0)
        nc.vector.memset(V[:, :, 129:130], 1.0)

        # ---- x staging: one tile per 512-token chunk, created lazily in
        # its DMA closure so ring order == issue order; projections look the
        # quarter tile up at emission time (always after its DMA) ----
        def x_chunks(xd, b, w, nm, t):
            f8 = F8[t]
            if f8:
                xr = xd.rearrange("(c two p) n -> p c two n", p=P, two=2)
            else:
                xr = xd.rearrange("(kc p) n -> p kc n", p=P)
            n0 = b * S
            tiles = {}
            dmas = []
            for j, t0 in enumerate(range(0, w, 512)):
                tw = min(512, w - t0)

                def mk(j=j, t0=t0, tw=tw):
                    if f8:
                        st = xpool.tile([P, DK // 2, 2, tw], dtx(t),
                                        tag="xst", name=f"{nm}_{j}")
                        nc.sync.dma_start(st,
                                          xr[:, :, :, n0 + t0:n0 + t0 + tw])
                    else:
                        st = xpool.tile([P, DK, tw], dtx(t), tag="xst",
                                        name=f"{nm}_{j}")
                        nc.sync.dma_start(st,
                                          xr[:, :, n0 + t0:n0 + t0 + tw])
                    tiles[j] = st
                dmas.append(mk)
            return tiles.get, dmas

        def proj_feat_group(x_get, q, tw, w_sb, dst, dst0, off=0, f8=False):
            # one ≤512-wide output tile of a feat-major projection from
            # quarter tile q (dst cols dst0+512q+off ...)
            x_sb = x_get(q)
            c0 = dst0 + q * 512 + off
            ps = ps_sc.tile([P, 512], dt.float32, tag="sc", name="ps_pr")
            if f8:
                for c in range(DK // 2):
                    nc.tensor.matmul(ps[:, :tw], lhsT=w_sb[:, c, :, :],
                                     rhs=x_sb[:, c, :, off:off + tw],
                                     start=(c == 0), stop=(c == DK // 2 - 1),
                                     perf_mode=PM)
            else:
                for kc in range(DK):
                    nc.tensor.matmul(ps[:, :tw], lhsT=w_sb[:, kc, :],
                                     rhs=x_sb[:, kc, off:off + tw],
                                     start=(kc == 0), stop=(kc == DK - 1))
            nc.vector.tensor_copy(out=dst[:, c0:c0 + tw], in_=ps[:, :tw])

        def proj_feat_split(x_get, q, tw, w_sb, dst, dst0, f8=False):
            # proj_feat_group split into two kc-halves to halve the PE
            # displacement when dropped into a chunk loop
            cell = {}
            HK = (DK // 2) // 2 if f8 else DK // 2

            def partA():
                cell["ps"] = ps_sc.tile([P, 512], dt.float32, tag="sc",
                                        name="ps_pr")
                ps = cell["ps"]
                if f8:
                    for c in range(HK):
                        nc.tensor.matmul(ps[:, :tw], lhsT=w_sb[:, c, :, :],
                                         rhs=x_get(q)[:, c, :, 0:tw],
                                         start=(c == 0), stop=False,
                                         perf_mode=PM)
                else:
                    for kc in range(HK):
                        nc.tensor.matmul(ps[:, :tw], lhsT=w_sb[:, kc, :],
                                         rhs=x_get(q)[:, kc, 0:tw],
                                         start=(kc == 0), stop=False)

            def partB():
                ps = cell["ps"]
                NK = DK // 2 if f8 else DK
                if f8:
                    for c in range(HK, NK):
                        nc.tensor.matmul(ps[:, :tw], lhsT=w_sb[:, c, :, :],
                                         rhs=x_get(q)[:, c, :, 0:tw],
                                         start=False, stop=(c == NK - 1),
                                         perf_mode=PM)
                else:
                    for kc in range(HK, NK):
                        nc.tensor.matmul(ps[:, :tw], lhsT=w_sb[:, kc, :],
                                         rhs=x_get(q)[:, kc, 0:tw],
                                         start=False, stop=(kc == NK - 1))
                nc.vector.tensor_copy(out=dst[:, dst0 + q * 512:
                                              dst0 + q * 512 + tw],
                                      in_=ps[:, :tw])
            return partA, partB

        def proj_tok_group(x_get, w_sb, b, kcl):
            # one 128-token chunk of the token-major V projection
            g = b * 16 + kcl
            x_sb = x_get(kcl // 4)
            t0 = (kcl % 4) * 128
            ps = ps_sc.tile([P, F], dt.float32, tag="sc", name="ps_v")
            if F8["v"]:
                for c in range(DK // 2):
                    nc.tensor.matmul(ps, lhsT=x_sb[:, c, :, t0:t0 + 128],
                                     rhs=w_sb[:, c, :, :],
                                     start=(c == 0), stop=(c == DK // 2 - 1),
                                     perf_mode=PM)
            else:
                for kc in range(DK):
                    nc.tensor.matmul(ps, lhsT=x_sb[:, kc, t0:t0 + 128],
                                     rhs=w_sb[:, kc, :],
                                     start=(kc == 0), stop=(kc == DK - 1))
            nc.vector.tensor_copy(out=V[:, g, 0:64], in_=ps[:, 0:64])
            nc.vector.tensor_copy(out=V[:, g, 65:129], in_=ps[:, 64:128])

        def tiles_of(w):
            return [(t0, min(512, w - t0)) for t0 in range(0, w, 512)]

        # filler queues: closures emitting one PE work group each, drained
        # into the attn chunk loop's exp-wait gaps. Tails drain only from
        # kcl>=2 so their transposes don't block PE on the qt-boundary
        # normalize chain.
        tail_q: list = []
        bulk_q: list = []

        def drain_one(kcl=2):
            if kcl >= 2 and tail_q:
                tail_q.pop(0)()
            elif bulk_q:
                bulk_q.pop(0)()
            elif kcl >= 2 and tail_q:
                tail_q.pop(0)()

        def qt_tail_items(b, qt, aos, split_dma=False, use_act=False,
                          use_sc=False):
            # the Wo tail split into 5 small filler items so it drains into
            # chunk-loop slack instead of stalling ACT at the qt boundary
            q0 = b * S + qt * 512
            cell = {}

            def item0():
                oT = otpool.tile([P, 512], DT_ATTN, tag="oT", name="oT")
                for qs in range(4):
                    tr = ps_wo.tile([P, P], DT_ATTN, tag="pw", name="tr")
                    nc.tensor.transpose(tr, aos[qs], ident)
                    nc.vector.tensor_copy(out=oT[:, qs * 128:(qs + 1) * 128],
                                          in_=tr)
                cell["oT"] = oT
                cell["ost"] = ospool.tile([P, 4, D], DT_OUT, tag="ost",
                                          name="ost")

            def mk_wo(sc4):
                def item():
                    oT, ost = cell["oT"], cell["ost"]
                    for half in range(2):
                        if use_sc:
                            pw = ps_sc.tile([P, 512], dt.float32, tag="sc",
                                            name="pw")
                        else:
                            pw = ps_wo.tile([P, 512], dt.float32, tag="pw",
                                            name="pw")
                        nc.tensor.matmul(
                            pw, lhsT=oT[:, sc4 * 128:(sc4 + 1) * 128],
                            rhs=wo_sb[:, half * 512:(half + 1) * 512])
                        dst = ost[:, sc4, half * 512:(half + 1) * 512]
                        if use_act and half == 0:
                            nc.scalar.copy(out=dst, in_=pw)
                        else:
                            nc.vector.tensor_copy(out=dst, in_=pw)
                    gs0 = q0 // 128
                    orr = out_d.rearrange("(g p) n -> p g n", p=P)
                    if split_dma:
                        nc.sync.dma_start(
                            orr[:, gs0 + sc4:gs0 + sc4 + 1, :],
                            cell["ost"][:, sc4:sc4 + 1, :])
                    elif sc4 == 3:
                        nc.sync.dma_start(orr[:, gs0:gs0 + 4, :],
                                          cell["ost"])
                return item
            return [item0, mk_wo(0), mk_wo(1), mk_wo(2), mk_wo(3)]

        # ---- schedule ----
        # b0: qt0's chunk loop doubles as the startup ramp — K/V quarter
        # DMAs + projections are embedded so exp starts after ~3 chunk DMAs.
        # b1: x DMAs and proj groups become filler drained through b0's attn.
        # qt Wo-tails are deferred one qt (front of the filler queue).
        def attn_qt(b, qt, embed=None, late=None, last=False,
                    finish_prev=None):
            late = late or {}
            q0 = b * S + qt * 512
            # start=True only on the FIRST matmul touching each bank: the
            # pending-zero region is the whole 2KB bank, so qs>0 chunk-0
            # matmuls must use start=False (their bytes are zero-filled by
            # qs0's mark; a second start=True would wipe qs0's result)
            av0 = ps_av.tile([P, 4, 65], dt.float32, tag="av0", name="av0")
            av1 = ps_av.tile([P, 4, 65], dt.float32, tag="av1", name="av1")
            pend = None
            for kcl in range(KC[b]):
                if embed is not None:
                    embed(kcl)
                if kcl in late:
                    late[kcl]()
                g = b * 16 + kcl
                k0 = b * S + kcl * 128
                sc2 = ps_sc.tile([P, 2, 512], dt.float32, tag="sc",
                                 name="sc2")
                nc.tensor.matmul(sc2[:, 0, :], lhsT=KT[0:64, k0:k0 + 128],
                                 rhs=QT[0:64, q0:q0 + 512])
                nc.tensor.matmul(sc2[:, 1, :], lhsT=KT[64:128, k0:k0 + 128],
                                 rhs=QT[64:128, q0:q0 + 512])
                at = apool.tile([P, 2, 512], DT_ATTN, tag="at", name="at")
                nc.scalar.activation(at.rearrange("p a n -> p (a n)"),
                                     sc2.rearrange("p a n -> p (a n)"),
                                     Exp, bias=mask_sb[:, g:g + 1],
                                     scale=0.125)
                # previous qt's last AV + normalize run after this qt's
                # first scores/exp so the boundary never starves ACT
                if kcl == 0 and finish_prev is not None:
                    finish_prev()
                # AV of the previous chunk runs after this chunk's scores so
                # PE never blocks on the current exp (1-deep pipeline)
                if pend is not None:
                    pend()
                if embed is None and kcl >= 1:
                    drain_one(kcl)

                def mk_av(at=at, g=g, st=(kcl == 0),
                          sp=(kcl == KC[b] - 1)):
                    for qs in range(4):
                        nc.tensor.matmul(
                            av0[:, qs, :],
                            lhsT=at[:, 0, qs * 128:(qs + 1) * 128],
                            rhs=V[:, g, 0:65], start=(st and qs == 0),
                            stop=sp)
                        nc.tensor.matmul(
                            av1[:, qs, :],
                            lhsT=at[:, 1, qs * 128:(qs + 1) * 128],
                            rhs=V[:, g, 65:130], start=(st and qs == 0),
                            stop=sp)
                pend = mk_av

            # deferred ending: last AV + normalize + tail push, executed
            # by the NEXT qt after its first scores/exp (or directly if
            # this is the final qt)

            def finish():
                pend()
                aoq = aopool.tile([P, 4, P], DT_ATTN, tag="ao", name="aoq")
                for h, av in ((0, av0), (1, av1)):
                    rc = rpool.tile([P, 4, 1], dt.float32, tag="rc",
                                    name="rc")
                    nc.vector.reciprocal(rc, av[:, :, 64:65])
                    in0, in1 = bass.broadcast_tensor_aps(av[:, :, 0:64],
                                                         rc[:, :, 0:1])
                    nc.vector.tensor_tensor(
                        out=aoq[:, :, h * 64:(h + 1) * 64], in0=in0,
                        in1=in1, op=mybir.AluOpType.mult)
                aos = [aoq[:, qs, :] for qs in range(4)]
                if last:
                    for it in qt_tail_items(b, qt, aos, split_dma=True,
                                            use_act=True, use_sc=True):
                        it()
                else:
                    # defer the Wo tail into the next qt's chunk loop
                    tail_q.extend(qt_tail_items(b, qt, aos))
            return finish

        # --- batch 0 startup ---
        gV0, pV0 = x_chunks(xtv_d, 0, KW[0], "xv0", "v")
        gK0, pK0 = x_chunks(xtk_d, 0, KW[0], "xk0", "k")
        gQ0, pQ0 = x_chunks(xtq_d, 0, S, "xq0", "q")
        nqKV = len(pV0)
        # prologue: DMA order = first-use order. mask is tiny but gates the
        # first exp via ACT program order, so it goes right after wk.
        dve_load(wk_sb, wre(wk_d, "k"), wsh("k"), dtx("k"), "wk")
        mask_raw = const.tile([P, B * 16], dt.float32, tag="mask_raw")
        nc.sync.dma_start(mask_raw, mask_d[:, :])
        nc.scalar.copy(out=mask_sb, in_=mask_raw)
        pK0[0]()
        dve_load(wq_sb, wre(wq_d, "q"), wsh("q"), dtx("q"), "wq")
        pQ0[0]()
        dve_load(wv_sb, wre(wv_d, "v"), wsh("v"), dtx("v"), "wv")
        pV0[0]()
        if nqKV > 1:
            pK0[1]()
            pV0[1]()
        late_consts()
        kt_tiles = tiles_of(KW[0])

        def embed0(kcl):
            if kcl % 4 == 0:
                q = kcl // 4
                if q + 2 < nqKV:
                    pK0[q + 2]()
                    pV0[q + 2]()
                if kcl == 4 or (KC[0] <= 4 and kcl == 0):
                    for t in range(1, 4):
                        pQ0[t]()
                if q > 0 and q < len(kt_tiles):
                    proj_feat_group(gK0, q, kt_tiles[q][1], wk_sb, KT, 0, f8=F8["k"])
            if kcl == 0:
                # minimal path to the first exps: K quarter 0 (arrives
                # first, small groups warm the PE p-state), then Q t0; the
                # V projections wait until kcl 1 (first needed by av(c0))
                proj_feat_group(gK0, 0, min(128, KW[0]), wk_sb, KT, 0,
                                f8=F8["k"])
                if KW[0] > 128:
                    proj_feat_group(gK0, 0, min(KW[0], 512) - 128, wk_sb,
                                    KT, 0, off=128, f8=F8["k"])
                proj_feat_group(gQ0, 0, 512, wq_sb, QT, 0, f8=F8["q"])
                if KC[0] == 1:
                    proj_tok_group(gV0, wv_sb, 0, 0)
            elif kcl == 1:
                for c in range(0, min(3, KC[0])):
                    proj_tok_group(gV0, wv_sb, 0, c)
            elif kcl + 1 < KC[0]:
                proj_tok_group(gV0, wv_sb, 0, kcl + 1)
            if KC[0] <= 8 and kcl == KC[0] - 1:
                for t in range(1, 4):
                    pQ0[t]()

        qA, qB = proj_feat_split(gQ0, 1, 512, wq_sb, QT, 0, f8=F8["q"])
        fin = attn_qt(0, 0, embed=embed0,
                      late={min(KC[0] - 2, 8): qA, min(KC[0] - 1, 9): qB})

        # --- batch 1 prefetch as filler (drained through b0 qt1-3) ---
        if B > 1:
            gV1, pV1 = x_chunks(xtv_d, 1, KW[1], "xv1", "v")
            gK1, pK1 = x_chunks(xtk_d, 1, KW[1], "xk1", "k")
            gQ1, pQ1 = x_chunks(xtq_d, 1, S, "xq1", "q")
            items = []
            kt1 = tiles_of(KW[1])
            nq1 = len(pV1)

            def v_pair(c0):
                def it():
                    for c in range(c0, min(c0 + 2, KC[1])):
                        proj_tok_group(gV1, wv_sb, 1, c)
                return it
            for j in range(nq1):
                items.append(pK1[j])
                items.append(pV1[j])
                if j >= 1:
                    jj = j - 1
                    kA, kB = proj_feat_split(gK1, jj, kt1[jj][1], wk_sb, KT,
                                             S, f8=F8["k"])
                    items.append(kA)
                    items.append(kB)
                    for c0 in range(4 * jj, min(4 * jj + 4, KC[1]), 2):
                        items.append(v_pair(c0))
            kA, kB = proj_feat_split(gK1, nq1 - 1, kt1[-1][1], wk_sb, KT, S,
                                     f8=F8["k"])
            items.append(kA)
            items.append(kB)
            for c0 in range(max(0, 4 * (nq1 - 1)), KC[1], 2):
                items.append(v_pair(c0))
            items.append(pQ1[0])
            items.append(pQ1[1])
            q0A, q0B = proj_feat_split(gQ1, 0, 512, wq_sb, QT, S,
                                       f8=F8["q"])
            items.append(q0A)
            items.append(q0B)
            bulk_q.extend(items)

        qA, qB = proj_feat_split(gQ0, 2, 512, wq_sb, QT, 0, f8=F8["q"])
        fin = attn_qt(0, 1, late={max(1, KC[0] // 2 - 1): qA,
                                  KC[0] // 2: qB}, finish_prev=fin)
        qA, qB = proj_feat_split(gQ0, 3, 512, wq_sb, QT, 0, f8=F8["q"])
        fin = attn_qt(0, 2, late={max(1, KC[0] // 2 - 1): qA,
                                  KC[0] // 2: qB}, finish_prev=fin)
        fin = attn_qt(0, 3, finish_prev=fin)
        # batch-1 attention reads KT/V/QT(b1): flush any un-drained
        # projection work before the first read is emitted
        while bulk_q:
            drain_one(0)
        if B > 1:
            pQ1[2]()
            pQ1[3]()
            mid = max(1, KC[1] // 2 - 1)
            for qt in range(4):
                late = {}
                if qt < 3:
                    qA, qB = proj_feat_split(gQ1, qt + 1, 512, wq_sb, QT, S,
                                             f8=F8["q"])
                    late = {mid: qA, mid + 1: qB}
                fin = attn_qt(1, qt, late=late, last=(qt == 3),
                              finish_prev=fin)
        fin()
        while tail_q or bulk_q:
            drain_one(2)

    _legalize_waits(nc)
    return nc


def _legalize_waits(nc):
    """This walrus build accepts at most ONE sync-wait command per
    instruction, while Tile emits up to a dozen (e.g. the kernel-tail
    drain). Legalize by splitting: excess waits are hoisted onto
    same-engine Drain instructions inserted immediately before the
    offender — same-engine program order makes this semantically
    identical. Patched module is served via nc.to_json_bytes."""
    import json as _json

    raw = nc.to_json_bytes()
    d = _json.loads(raw)
    template = None
    for fn in d.get("functions", []):
        for blk in fn.get("blocks", []):
            for inst in blk.get("instructions", []):
                if inst.get("opcode") == "Drain":
                    template = inst
                    break
            if template:
                break
        if template:
            break
    assert template is not None, "no Drain template found"

    counter = [0]

    def carrier(engine, wait):
        counter[0] += 1
        c = _json.loads(_json.dumps(template))
        c["name"] = f"I-waitfix-{counter[0]}"
        c["engine"] = engine
        c["sync_info"] = {"on_update": [], "on_wait": [wait]}
        c["ins"] = []
        c["outs"] = []
        return c

    nfix = 0
    for fn in d.get("functions", []):
        for blk in fn.get("blocks", []):
            out = []
            for inst in blk.get("instructions", []):
                si = inst.get("sync_info")
                waits = (si or {}).get("on_wait") or []
                if len(waits) > 1:
                    for w in waits[:-1]:
                        out.append(carrier(inst["engine"], w))
                    si["on_wait"] = [waits[-1]]
                    nfix += 1
                out.append(inst)
            blk["instructions"] = out

    patched = _json.dumps(d).encode()
    nc.to_json_bytes = lambda: patched


def _prep_host(queries, keys, values, Wq, Wk, Wv, Wo, valid_lens, cfg):
    np_map = {"bfloat16": ml_dtypes.bfloat16, "float32": np.float32,
              "float16": np.float16}
    p8 = str(cfg.get("proj8", "0"))
    F8 = {"q": p8 in ("1", "qk", "q"), "k": p8 in ("1", "qk", "q"),
          "v": p8 == "1"}
    np_in = np_map[cfg["dt_in"]]

    def np_x(t):
        return ml_dtypes.float8_e4m3 if F8[t] else np_in
    L = [int(valid_lens[0]), int(valid_lens[1])]
    KC = tuple(min(16, (l + 127) // 128) for l in L)

    def t2(x, t):  # (B,S,D) -> (D, B*S)
        return np.ascontiguousarray(
            np.asarray(x, np.float32).reshape(NT, D).T).astype(np_x(t))

    xtq, xtk, xtv = t2(queries, "q"), t2(keys, "k"), t2(values, "v")
    maskt = np.full((P, B * 16), NEG, np.float32)
    for b in range(B):
        for c in range(16):
            ks = c * 128 + np.arange(P)
            maskt[:, b * 16 + c] = np.where(ks < L[b], 0.0, NEG)

    Wq = np.asarray(Wq, np.float32)
    Wk = np.asarray(Wk, np.float32)
    Wv = np.asarray(Wv, np.float32)
    Wo = np.asarray(Wo, np.float32)
    in_maps = []
    for c in range(N_CORES):
        cs = slice(c * F, (c + 1) * F)
        in_maps.append({
            "xtq": xtq, "xtk": xtk, "xtv": xtv,
            "wq": np.ascontiguousarray(Wq[:, cs]).astype(np_x("q")),
            "wk": np.ascontiguousarray(Wk[:, cs]).astype(np_x("k")),
            "wv": np.ascontiguousarray(Wv[:, cs]).astype(np_x("v")),
            "wo": np.ascontiguousarray(Wo[cs, :]).astype(np_in),
            "maskt": maskt,
        })
    return KC, in_maps


DEFAULT_CFG = {"dt_in": "float16", "dt_attn": "float16", "dt_out": "float16",
               "proj8": "0"}

LAST_RESULTS = None


def kernel(queries, keys, values, Wq, Wk, Wv, Wo, valid_lens):
    global LAST_RESULTS
    from concourse.bass_utils import run_bass_kernel_spmd

    cfg = dict(DEFAULT_CFG)
    if os.environ.get("MHA_CFG"):
        for kv in os.environ["MHA_CFG"].split(","):
            k, v = kv.split("=")
            cfg[k] = v

    KC, in_maps = _prep_host(queries, keys, values, Wq, Wk, Wv, Wo,
                             valid_lens, cfg)
    key = (KC, tuple(sorted(cfg.items())))
    if key not in _CACHE:
        _CACHE[key] = _build_program(KC, cfg)
    nc = _CACHE[key]

    trace = bool(os.environ.get("MHA_TRACE"))
    res = run_bass_kernel_spmd(nc, in_maps, core_ids=list(range(N_CORES)),
                               trace=trace)
    LAST_RESULTS = res
    acc = np.zeros((NT, D), np.float32)
    for r in res.results:
        acc += np.asarray(r["out_part"], np.float32)
    return acc.reshape(B, S, D)
